# revision 9
# baseline (speedup 1.0000x reference)
"""Trainium2 Bass kernel for CrossAttentionFusion (seq_len=1 cross-attn + LN + sigmoid gate).

Contract: kernel(**inputs) takes the FULL unsharded inputs (B=32768, D=1024),
shards the batch across 8 NeuronCores (pure data parallel, weights replicated),
runs a Bass/Tile kernel per core, and returns the full (fused_feature,
gate_weight) tuple, both [B, D] float32.

Math (matches the reference exactly):
  attn_g2s = smiles @ (Wo_gs @ Wv_gs).T + (Wo_gs @ bv_gs + bo_gs)
  attn_s2g = graph  @ (Wo_sg @ Wv_sg).T + (Wo_sg @ bv_sg + bo_sg)
  fused_g  = LN(graph + attn_g2s) * g_g + b_g
  fused_s  = LN(smiles + attn_s2g) * g_s + b_s
  gate     = sigmoid(fused_g @ Wg[:, :D].T + fused_s @ Wg[:, D:].T + bg)
  out      = gate * fused_g + (1 - gate) * fused_s
"""

import sys
import os

sys.path.insert(0, "/opt/trn_rl_repo")

import numpy as np

B_FULL = 32768
D = 1024
N_CORES = 8
LN_EPS = 1e-5

_CACHE = {}


def _build(rows_per_core: int, d: int, nonzero_attn_bias: bool,
           nonzero_gate_bias: bool, ln_identity: bool):
    """Build + compile the per-core Bass program. Returns (nc, names)."""
    import concourse.bass as bass
    import concourse.bacc as bacc
    import concourse.mybir as mybir
    import concourse.tile as tile
    from concourse.masks import make_identity

    F32 = mybir.dt.float32
    F32R = mybir.dt.float32r
    BF16 = mybir.dt.bfloat16
    AT = mybir.ActivationFunctionType
    OP = mybir.AluOpType

    R = rows_per_core
    assert R % 128 == 0 and d == 1024
    NB = R // 128           # number of 128-row batch tiles
    KS = d // 128           # 8 contraction slabs
    NH = d // 512           # 2 free-dim halves

    nc = bacc.Bacc("TRN2", target_bir_lowering=False, debug=False,
                   num_devices=N_CORES)

    xg_d = nc.declare_dram_parameter("xg", [R, d], F32R, isOutput=False)
    xs_d = nc.declare_dram_parameter("xs", [R, d], F32R, isOutput=False)
    # combined attention weights, layout [d_in, d_out] (i.e. Wc.T), f32r typed
    wgs_d = nc.declare_dram_parameter("wgs", [d, d], F32R, isOutput=False)
    wsg_d = nc.declare_dram_parameter("wsg", [d, d], F32R, isOutput=False)
    # gate weights (gamma folded in on host), layout [d_in, d_out], bf16
    wg1_d = nc.declare_dram_parameter("wg1", [d, d], BF16, isOutput=False)
    wg2_d = nc.declare_dram_parameter("wg2", [d, d], BF16, isOutput=False)
    if nonzero_attn_bias:
        bcg_d = nc.declare_dram_parameter("bcg", [128, d], F32, isOutput=False)
        bcs_d = nc.declare_dram_parameter("bcs", [128, d], F32, isOutput=False)
    if nonzero_gate_bias:
        bgg_d = nc.declare_dram_parameter("bgg", [128, d], F32, isOutput=False)
    if not ln_identity:
        gbg_d = nc.declare_dram_parameter("gbg", [128, 2 * d], F32, isOutput=False)
        gbs_d = nc.declare_dram_parameter("gbs", [128, 2 * d], F32, isOutput=False)
    outf_d = nc.declare_dram_parameter("outf", [R, d], F32, isOutput=True)
    outg_d = nc.declare_dram_parameter("outg", [R, d], F32, isOutput=True)

    with tile.TileContext(nc) as tc:
        with (
            tc.tile_pool(name="wpool", bufs=1) as wp,
            tc.tile_pool(name="io", bufs=2) as io,
            tc.tile_pool(name="xt", bufs=2) as xtp,
            tc.tile_pool(name="work", bufs=2) as wk,
            tc.tile_pool(name="st", bufs=3) as st,
            tc.tile_pool(name="scr", bufs=1) as scr,
            tc.tile_pool(name="ptra", bufs=2, space=bass.MemorySpace.PSUM) as ptr,
            tc.tile_pool(name="ptrb", bufs=2, space=bass.MemorySpace.PSUM) as ptrb,
            tc.tile_pool(name="pmma", bufs=3, space=bass.MemorySpace.PSUM) as pmm,
            tc.tile_pool(name="pmmg", bufs=1, space=bass.MemorySpace.PSUM) as pmmg,
        ):
            ident = wp.tile([128, 128], F32)
            make_identity(nc, ident[:])
            identr = wp.tile([128, 128], F32R)
            nc.vector.tensor_copy(identr[:], ident[:])
            identb = wp.tile([128, 128], BF16)
            nc.vector.tensor_copy(identb[:], ident[:])

            # resident weights: 8 slabs of [128, d] each, slab k at cols [k*d,(k+1)*d)
            wgs = wp.tile([128, KS * d], F32R)
            wsg = wp.tile([128, KS * d], F32R)
            wg1 = wp.tile([128, KS * d], BF16)
            wg2 = wp.tile([128, KS * d], BF16)
            for k in range(KS):
                nc.sync.dma_start(wgs[:, k * d:(k + 1) * d], wgs_d[k * 128:(k + 1) * 128, :])
                nc.sync.dma_start(wsg[:, k * d:(k + 1) * d], wsg_d[k * 128:(k + 1) * 128, :])
                nc.sync.dma_start(wg1[:, k * d:(k + 1) * d], wg1_d[k * 128:(k + 1) * 128, :])
                nc.sync.dma_start(wg2[:, k * d:(k + 1) * d], wg2_d[k * 128:(k + 1) * 128, :])
            if nonzero_attn_bias:
                bcg = wp.tile([128, d], F32)
                bcs = wp.tile([128, d], F32)
                nc.sync.dma_start(bcg[:], bcg_d[:])
                nc.sync.dma_start(bcs[:], bcs_d[:])
            if nonzero_gate_bias:
                bgg = wp.tile([128, d], F32)
                nc.sync.dma_start(bgg[:], bgg_d[:])
            if not ln_identity:
                gbg = wp.tile([128, 2 * d], F32)
                gbs = wp.tile([128, 2 * d], F32)
                nc.sync.dma_start(gbg[:], gbg_d[:])
                nc.sync.dma_start(gbs[:], gbs_d[:])

            def phase_a(i):
                rows = slice(i * 128, (i + 1) * 128)
                xg = io.tile([128, d], F32R, tag="xg")
                xs = io.tile([128, d], F32R, tag="xs")
                nc.sync.dma_start(xg[:], xg_d[rows, :])
                nc.sync.dma_start(xs[:], xs_d[rows, :])

                # transpose inputs via PE; xT slab k at cols [k*128,(k+1)*128)
                xgT = xtp.tile([128, KS * 128], F32R, tag="xgT")
                xsT = xtp.tile([128, KS * 128], F32R, tag="xsT")
                for src_, dstT in ((xg, xgT), (xs, xsT)):
                    for h in range(2):
                        pt = ptr.tile([128, 512], F32R, tag="pt")
                        for b in range(4):
                            k = h * 4 + b
                            nc.tensor.matmul(
                                pt[:, b * 128:(b + 1) * 128],
                                src_[:, k * 128:(k + 1) * 128],
                                identr[:],
                                is_transpose=True,
                                start=(b == 0), stop=(b == 3),
                                skip_group_check=True,
                            )
                        nc.scalar.copy(dstT[:, h * 512:(h + 1) * 512], pt[:])

                # attention matmuls (f32r) accumulating in PSUM, per 512-col half;
                # residual STT consumes each half as soon as it completes.
                S4 = st.tile([128, 4], F32, tag="S4")
                Q = st.tile([128, 2], F32, tag="Q")
                rg = wk.tile([128, d], F32, tag="rg")
                rs_ = wk.tile([128, d], F32, tag="rs_")
                for nh in range(NH):
                    cols = slice(nh * 512, (nh + 1) * 512)
                    pah_g = pmm.tile([128, 512], F32, tag="pah")
                    for k in range(KS):
                        nc.tensor.matmul(
                            pah_g[:],
                            xsT[:, k * 128:(k + 1) * 128],
                            wgs[:, k * d + nh * 512: k * d + nh * 512 + 512],
                            start=(k == 0), stop=(k == KS - 1),
                        )
                    pah_s = pmm.tile([128, 512], F32, tag="pah")
                    for k in range(KS):
                        nc.tensor.matmul(
                            pah_s[:],
                            xgT[:, k * 128:(k + 1) * 128],
                            wsg[:, k * d + nh * 512: k * d + nh * 512 + 512],
                            start=(k == 0), stop=(k == KS - 1),
                        )
                    nc.vector.scalar_tensor_tensor(
                        rg[:, cols], pah_g[:], 1.0, xg[:, cols].bitcast(F32), op0=OP.mult,
                        op1=OP.add, accum_out=S4[:, 2 * nh:2 * nh + 1])
                    nc.vector.scalar_tensor_tensor(
                        rs_[:, cols], pah_s[:], 1.0, xs[:, cols].bitcast(F32), op0=OP.mult,
                        op1=OP.add, accum_out=S4[:, 2 * nh + 1:2 * nh + 2])
                # S[:, {g,s}] = sum over halves
                S = st.tile([128, 2], F32, tag="S")
                nc.vector.tensor_tensor(S[:], S4[:, 0:2], S4[:, 2:4], op=OP.add)
                if nonzero_attn_bias:
                    nc.vector.tensor_tensor(rg[:], rg[:], bcg[:], op=OP.add)
                    nc.vector.tensor_tensor(rs_[:], rs_[:], bcs[:], op=OP.add)
                    nc.vector.tensor_reduce(S[:, 0:1], rg[:], axis=mybir.AxisListType.X, op=OP.add)
                    nc.vector.tensor_reduce(S[:, 1:2], rs_[:], axis=mybir.AxisListType.X, op=OP.add)

                # sum of squares via ACT (fused square + accumulate)
                sq = scr.tile([128, d], F32, tag="sq")
                nc.scalar.activation(sq[:], rg[:], AT.Square, accum_out=Q[:, 0:1])
                nc.scalar.activation(sq[:], rs_[:], AT.Square, accum_out=Q[:, 1:2])

                # layernorm scalars (DVE only, quake rsqrt + 2 Newton iters)
                I32 = mybir.dt.int32
                mu = st.tile([128, 2], F32, tag="mu")
                vv = st.tile([128, 2], F32, tag="vv")
                m2 = st.tile([128, 2], F32, tag="m2")
                hh = st.tile([128, 2], I32, tag="hh")
                yy = st.tile([128, 2], F32, tag="yy")
                aa = st.tile([128, 2], F32, tag="aa")
                bb = st.tile([128, 2], F32, tag="bb")
                cx = st.tile([128, 2], F32, tag="cx")
                ri = st.tile([128, 2], F32, tag="ri")
                nb = st.tile([128, 2], F32, tag="nb")
                inv_n = 1.0 / d
                nc.vector.tensor_scalar(mu[:], S[:], inv_n, None, op0=OP.mult)
                nc.vector.tensor_scalar(vv[:], Q[:], inv_n, None, op0=OP.mult)
                nc.vector.tensor_tensor(m2[:], mu[:], mu[:], op=OP.mult)
                nc.vector.scalar_tensor_tensor(
                    vv[:], vv[:], float(LN_EPS), m2[:], op0=OP.add, op1=OP.subtract)
                nc.vector.tensor_scalar(hh[:], vv[:].bitcast(I32), 1,
                                        None, op0=OP.arith_shift_right)
                nc.vector.tensor_scalar(hh[:], hh[:], -1, 0x5F3759DF,
                                        op0=OP.mult, op1=OP.add)
                nc.vector.tensor_copy(yy[:], hh[:].bitcast(F32))
                for _ in range(2):
                    nc.vector.tensor_tensor(aa[:], yy[:], yy[:], op=OP.mult)
                    nc.vector.tensor_tensor(bb[:], aa[:], vv[:], op=OP.mult)
                    nc.vector.tensor_scalar(cx[:], bb[:], -0.5, 1.5,
                                            op0=OP.mult, op1=OP.add)
                    nc.vector.tensor_tensor(yy[:], yy[:], cx[:], op=OP.mult)
                nc.vector.scalar_tensor_tensor(
                    nb[:], mu[:], -1.0, yy[:], op0=OP.mult, op1=OP.mult)
                return dict(i=i, rg=rg, rs_=rs_, ri=yy, nb=nb)

            def phase_b(ctx):
                i, rg, rs_, ri, nb = ctx["i"], ctx["rg"], ctx["rs_"], ctx["ri"], ctx["nb"]
                rows = slice(i * 128, (i + 1) * 128)
                tg = wk.tile([128, d], F32, tag="tg")
                ts = wk.tile([128, d], F32, tag="ts")
                nc.scalar.activation(tg[:], rg[:], AT.Identity,
                                     bias=nb[:, 0:1], scale=ri[:, 0:1])
                nc.scalar.activation(ts[:], rs_[:], AT.Identity,
                                     bias=nb[:, 1:2], scale=ri[:, 1:2])
                if ln_identity:
                    yg, ys = tg, ts
                else:
                    yg = wk.tile([128, d], F32, tag="yg")
                    ys = wk.tile([128, d], F32, tag="ys")
                    nc.vector.scalar_tensor_tensor(
                        yg[:], tg[:], 1.0, gbg[:, 0:d], op0=OP.mult, op1=OP.mult)
                    nc.vector.tensor_tensor(yg[:], yg[:], gbg[:, d:2 * d], op=OP.add)
                    nc.vector.scalar_tensor_tensor(
                        ys[:], ts[:], 1.0, gbs[:, 0:d], op0=OP.mult, op1=OP.mult)
                    nc.vector.tensor_tensor(ys[:], ys[:], gbs[:, d:2 * d], op=OP.add)

                # transpose t in bf16 (cast first on DVE; 8 blocks fit one bank)
                tgb = scr.tile([128, d], BF16, tag="tgb")
                tsb = scr.tile([128, d], BF16, tag="tsb")
                nc.vector.tensor_copy(tgb[:], tg[:])
                nc.vector.tensor_copy(tsb[:], ts[:])
                tgT = xtp.tile([128, KS * 128], BF16, tag="tgT")
                tsT = xtp.tile([128, KS * 128], BF16, tag="tsT")
                for src_, dstT in ((tgb, tgT), (tsb, tsT)):
                    pt2 = ptrb.tile([128, KS * 128], BF16, tag="pt2")
                    for b in range(KS):
                        nc.tensor.matmul(
                            pt2[:, b * 128:(b + 1) * 128],
                            src_[:, b * 128:(b + 1) * 128],
                            identb[:],
                            is_transpose=True,
                            start=(b == 0), stop=(b == KS - 1),
                            skip_group_check=True,
                        )
                    nc.scalar.copy(dstT[:], pt2[:])

                # gate matmul (bf16)
                g = io.tile([128, d], F32, tag="g")
                for nh in range(NH):
                    cols = slice(nh * 512, (nh + 1) * 512)
                    pgl = pmmg.tile([128, 512], F32, tag="pgl")
                    for k in range(KS):
                        nc.tensor.matmul(
                            pgl[:],
                            tgT[:, k * 128:(k + 1) * 128],
                            wg1[:, k * d + nh * 512: k * d + nh * 512 + 512],
                            start=(k == 0), stop=False,
                        )
                    for k in range(KS):
                        nc.tensor.matmul(
                            pgl[:],
                            tsT[:, k * 128:(k + 1) * 128],
                            wg2[:, k * d + nh * 512: k * d + nh * 512 + 512],
                            start=False, stop=(k == KS - 1),
                        )
                    if nonzero_gate_bias:
                        gl = scr.tile([128, 512], F32, tag="gl")
                        nc.vector.tensor_tensor(gl[:], pgl[:], bgg[:, cols], op=OP.add)
                        nc.scalar.activation(g[:, cols], gl[:], AT.Sigmoid)
                    else:
                        nc.scalar.activation(g[:, cols], pgl[:], AT.Sigmoid)

                # blend: out = ys + g * (yg - ys)
                dd = wk.tile([128, d], F32, tag="dd")
                pp = scr.tile([128, d], F32, tag="pp")
                outf = io.tile([128, d], F32, tag="outf")
                nc.vector.tensor_tensor(dd[:], yg[:], ys[:], op=OP.subtract)
                nc.vector.tensor_tensor(pp[:], dd[:], g[:], op=OP.mult)
                nc.vector.tensor_tensor(outf[:], pp[:], ys[:], op=OP.add)

                nc.sync.dma_start(outf_d[rows, :], outf[:])
                nc.sync.dma_start(outg_d[rows, :], g[:])

            pending = None
            for i in range(NB):
                ctx = phase_a(i)
                if pending is not None:
                    phase_b(pending)
                pending = ctx
            phase_b(pending)

    nc.compile()
    return nc


def _prep_host(inputs):
    """Host-side weight algebra. Returns dict of device arrays + flags."""
    import ml_dtypes

    f = lambda k: np.asarray(inputs[k], dtype=np.float32)
    Wv_gs, Wo_gs = f("Wv_gs"), f("Wo_gs")
    Wv_sg, Wo_sg = f("Wv_sg"), f("Wo_sg")
    bv_gs, bo_gs = f("bv_gs"), f("bo_gs")
    bv_sg, bo_sg = f("bv_sg"), f("bo_sg")
    Wg, bg = f("Wg"), f("bg")
    g_g, b_g = f("ln_g_gamma"), f("ln_g_beta")
    g_s, b_s = f("ln_s_gamma"), f("ln_s_beta")

    Wcgs = Wo_gs @ Wv_gs            # [D, D]; attn_g2s = smiles @ Wcgs.T + bcgs
    Wcsg = Wo_sg @ Wv_sg
    bcgs = Wo_gs @ bv_gs + bo_gs
    bcsg = Wo_sg @ bv_sg + bo_sg

    ln_identity = (np.all(g_g == 1.0) and np.all(b_g == 0.0)
                   and np.all(g_s == 1.0) and np.all(b_s == 0.0))

    # gate weights with gamma folded in (rows of Wg.T scale by gamma of the
    # un-gamma'd LN output t), bias collects beta contributions
    Wg1 = Wg[:, :D]                  # [D_out, D_in]
    Wg2 = Wg[:, D:]
    Wg1T = (Wg1 * g_g[None, :]).T.copy()     # [D_in, D_out]
    Wg2T = (Wg2 * g_s[None, :]).T.copy()
    bg_eff = bg + Wg1 @ b_g + Wg2 @ b_s

    out = {
        "wgs": Wcgs.T.copy(),        # [d_in, d_out] f32 (f32r-typed on device)
        "wsg": Wcsg.T.copy(),
        "wg1": Wg1T.astype(ml_dtypes.bfloat16),
        "wg2": Wg2T.astype(ml_dtypes.bfloat16),
    }
    flags = {
        "nonzero_attn_bias": bool(np.any(bcgs != 0) or np.any(bcsg != 0)),
        "nonzero_gate_bias": bool(np.any(bg_eff != 0)),
        "ln_identity": bool(ln_identity),
    }
    if flags["nonzero_attn_bias"]:
        out["bcg"] = np.tile(bcgs[None, :], (128, 1)).astype(np.float32)
        out["bcs"] = np.tile(bcsg[None, :], (128, 1)).astype(np.float32)
    if flags["nonzero_gate_bias"]:
        out["bgg"] = np.tile(bg_eff[None, :], (128, 1)).astype(np.float32)
    if not flags["ln_identity"]:
        out["gbg"] = np.tile(np.concatenate([g_g, b_g])[None, :], (128, 1)).astype(np.float32)
        out["gbs"] = np.tile(np.concatenate([g_s, b_s])[None, :], (128, 1)).astype(np.float32)
    return out, flags


def _run(inputs, rows_per_core, trace=False):
    from concourse.bass_utils import run_bass_kernel_spmd

    graph = np.ascontiguousarray(np.asarray(inputs["graph_embedding"], dtype=np.float32))
    smiles = np.ascontiguousarray(np.asarray(inputs["smiles_embedding"], dtype=np.float32))
    B = graph.shape[0]
    assert B == rows_per_core * N_CORES

    wmap, flags = _prep_host(inputs)
    key = (rows_per_core, D, flags["nonzero_attn_bias"],
           flags["nonzero_gate_bias"], flags["ln_identity"])
    if key not in _CACHE:
        _CACHE[key] = _build(rows_per_core, D, flags["nonzero_attn_bias"],
                             flags["nonzero_gate_bias"], flags["ln_identity"])
    nc = _CACHE[key]

    in_maps = []
    for c in range(N_CORES):
        rows = slice(c * rows_per_core, (c + 1) * rows_per_core)
        m = {"xg": graph[rows], "xs": smiles[rows]}
        m.update(wmap)
        in_maps.append(m)

    res = run_bass_kernel_spmd(nc, in_maps, core_ids=list(range(N_CORES)),
                               trace=trace)
    fused = np.concatenate([r["outf"] for r in res.results], axis=0)
    gate = np.concatenate([r["outg"] for r in res.results], axis=0)
    return (fused, gate), res


def kernel(**inputs):
    (fused, gate), _ = _run(inputs, B_FULL // N_CORES)
    return fused, gate


# revision 10
# speedup vs baseline: 1.0295x; 1.0295x over previous
"""Trainium2 Bass kernel for CrossAttentionFusion (seq_len=1 cross-attn + LN + sigmoid gate).

Contract: kernel(**inputs) takes the FULL unsharded inputs (B=32768, D=1024),
shards the batch across 8 NeuronCores (pure data parallel, weights replicated),
runs a Bass/Tile kernel per core, and returns the full (fused_feature,
gate_weight) tuple, both [B, D] float32.

Math (matches the reference exactly):
  attn_g2s = smiles @ (Wo_gs @ Wv_gs).T + (Wo_gs @ bv_gs + bo_gs)
  attn_s2g = graph  @ (Wo_sg @ Wv_sg).T + (Wo_sg @ bv_sg + bo_sg)
  fused_g  = LN(graph + attn_g2s) * g_g + b_g
  fused_s  = LN(smiles + attn_s2g) * g_s + b_s
  gate     = sigmoid(fused_g @ Wg[:, :D].T + fused_s @ Wg[:, D:].T + bg)
  out      = gate * fused_g + (1 - gate) * fused_s
"""

import sys
import os

sys.path.insert(0, "/opt/trn_rl_repo")

import numpy as np

B_FULL = 32768
D = 1024
N_CORES = 8
LN_EPS = 1e-5

_CACHE = {}


def _build(rows_per_core: int, d: int, nonzero_attn_bias: bool,
           nonzero_gate_bias: bool, ln_identity: bool):
    """Build + compile the per-core Bass program. Returns (nc, names)."""
    import concourse.bass as bass
    import concourse.bacc as bacc
    import concourse.mybir as mybir
    import concourse.tile as tile
    from concourse.masks import make_identity

    F32 = mybir.dt.float32
    F32R = mybir.dt.float32r
    BF16 = mybir.dt.bfloat16
    AT = mybir.ActivationFunctionType
    OP = mybir.AluOpType

    R = rows_per_core
    assert R % 128 == 0 and d == 1024
    NB = R // 128           # number of 128-row batch tiles
    KS = d // 128           # 8 contraction slabs
    NH = d // 512           # 2 free-dim halves

    nc = bacc.Bacc("TRN2", target_bir_lowering=False, debug=False,
                   num_devices=N_CORES)

    xg_d = nc.declare_dram_parameter("xg", [R, d], F32R, isOutput=False)
    xs_d = nc.declare_dram_parameter("xs", [R, d], F32R, isOutput=False)
    # combined attention weights, layout [d_in, d_out] (i.e. Wc.T), f32r typed
    wgs_d = nc.declare_dram_parameter("wgs", [d, d], F32R, isOutput=False)
    wsg_d = nc.declare_dram_parameter("wsg", [d, d], F32R, isOutput=False)
    # gate weights (gamma folded in on host), layout [d_in, d_out], bf16
    wg1_d = nc.declare_dram_parameter("wg1", [d, d], BF16, isOutput=False)
    wg2_d = nc.declare_dram_parameter("wg2", [d, d], BF16, isOutput=False)
    if nonzero_attn_bias:
        bcg_d = nc.declare_dram_parameter("bcg", [128, d], F32, isOutput=False)
        bcs_d = nc.declare_dram_parameter("bcs", [128, d], F32, isOutput=False)
    if nonzero_gate_bias:
        bgg_d = nc.declare_dram_parameter("bgg", [128, d], F32, isOutput=False)
    if not ln_identity:
        gbg_d = nc.declare_dram_parameter("gbg", [128, 2 * d], F32, isOutput=False)
        gbs_d = nc.declare_dram_parameter("gbs", [128, 2 * d], F32, isOutput=False)
    outf_d = nc.declare_dram_parameter("outf", [R, d], F32, isOutput=True)
    outg_d = nc.declare_dram_parameter("outg", [R, d], F32, isOutput=True)

    with tile.TileContext(nc) as tc:
        with (
            tc.tile_pool(name="wpool", bufs=1) as wp,
            tc.tile_pool(name="io", bufs=2) as io,
            tc.tile_pool(name="xt", bufs=2) as xtp,
            tc.tile_pool(name="work", bufs=2) as wk,
            tc.tile_pool(name="st", bufs=3) as st,
            tc.tile_pool(name="scr", bufs=1) as scr,
            tc.tile_pool(name="ptra", bufs=2, space=bass.MemorySpace.PSUM) as ptr,
            tc.tile_pool(name="ptrb", bufs=2, space=bass.MemorySpace.PSUM) as ptrb,
            tc.tile_pool(name="pmma", bufs=3, space=bass.MemorySpace.PSUM) as pmm,
            tc.tile_pool(name="pmmg", bufs=1, space=bass.MemorySpace.PSUM) as pmmg,
        ):
            ident = wp.tile([128, 128], F32)
            make_identity(nc, ident[:])
            identr = wp.tile([128, 128], F32R)
            nc.vector.tensor_copy(identr[:], ident[:])
            identb = wp.tile([128, 128], BF16)
            nc.vector.tensor_copy(identb[:], ident[:])

            # resident weights: 8 slabs of [128, d] each, slab k at cols [k*d,(k+1)*d)
            wgs = wp.tile([128, KS * d], F32R)
            wsg = wp.tile([128, KS * d], F32R)
            wg1 = wp.tile([128, KS * d], BF16)
            wg2 = wp.tile([128, KS * d], BF16)
            for k in range(KS):
                nc.sync.dma_start(wgs[:, k * d:(k + 1) * d], wgs_d[k * 128:(k + 1) * 128, :])
                nc.sync.dma_start(wsg[:, k * d:(k + 1) * d], wsg_d[k * 128:(k + 1) * 128, :])
                nc.sync.dma_start(wg1[:, k * d:(k + 1) * d], wg1_d[k * 128:(k + 1) * 128, :])
                nc.sync.dma_start(wg2[:, k * d:(k + 1) * d], wg2_d[k * 128:(k + 1) * 128, :])
            if nonzero_attn_bias:
                bcg = wp.tile([128, d], F32)
                bcs = wp.tile([128, d], F32)
                nc.sync.dma_start(bcg[:], bcg_d[:])
                nc.sync.dma_start(bcs[:], bcs_d[:])
            if nonzero_gate_bias:
                bgg = wp.tile([128, d], F32)
                nc.sync.dma_start(bgg[:], bgg_d[:])
            if not ln_identity:
                gbg = wp.tile([128, 2 * d], F32)
                gbs = wp.tile([128, 2 * d], F32)
                nc.sync.dma_start(gbg[:], gbg_d[:])
                nc.sync.dma_start(gbs[:], gbs_d[:])

            def phase_a(i):
                rows = slice(i * 128, (i + 1) * 128)
                xg = io.tile([128, d], F32R, tag="xg")
                xs = io.tile([128, d], F32R, tag="xs")
                nc.sync.dma_start(xg[:], xg_d[rows, :])
                nc.sync.dma_start(xs[:], xs_d[rows, :])

                # transpose inputs via PE; xT slab k at cols [k*128,(k+1)*128)
                xgT = xtp.tile([128, KS * 128], F32R, tag="xgT")
                xsT = xtp.tile([128, KS * 128], F32R, tag="xsT")
                for src_, dstT in ((xg, xgT), (xs, xsT)):
                    for h in range(2):
                        pt = ptr.tile([128, 512], F32R, tag="pt")
                        for b in range(4):
                            k = h * 4 + b
                            nc.tensor.matmul(
                                pt[:, b * 128:(b + 1) * 128],
                                src_[:, k * 128:(k + 1) * 128],
                                identr[:],
                                is_transpose=True,
                                start=(b == 0), stop=(b == 3),
                                skip_group_check=True,
                            )
                        nc.scalar.copy(dstT[:, h * 512:(h + 1) * 512], pt[:])

                # attention matmuls (f32r) accumulating in PSUM, per 512-col half;
                # residual STT consumes each half as soon as it completes.
                S4 = st.tile([128, 4], F32, tag="S4")
                Q = st.tile([128, 2], F32, tag="Q")
                rg = wk.tile([128, d], F32, tag="rg")
                rs_ = wk.tile([128, d], F32, tag="rs_")
                for nh in range(NH):
                    cols = slice(nh * 512, (nh + 1) * 512)
                    pah_g = pmm.tile([128, 512], F32, tag="pah")
                    for k in range(KS):
                        nc.tensor.matmul(
                            pah_g[:],
                            xsT[:, k * 128:(k + 1) * 128],
                            wgs[:, k * d + nh * 512: k * d + nh * 512 + 512],
                            start=(k == 0), stop=(k == KS - 1),
                        )
                    pah_s = pmm.tile([128, 512], F32, tag="pah")
                    for k in range(KS):
                        nc.tensor.matmul(
                            pah_s[:],
                            xgT[:, k * 128:(k + 1) * 128],
                            wsg[:, k * d + nh * 512: k * d + nh * 512 + 512],
                            start=(k == 0), stop=(k == KS - 1),
                        )
                    nc.vector.scalar_tensor_tensor(
                        rg[:, cols], pah_g[:], 1.0, xg[:, cols].bitcast(F32), op0=OP.mult,
                        op1=OP.add, accum_out=S4[:, 2 * nh:2 * nh + 1])
                    nc.vector.scalar_tensor_tensor(
                        rs_[:, cols], pah_s[:], 1.0, xs[:, cols].bitcast(F32), op0=OP.mult,
                        op1=OP.add, accum_out=S4[:, 2 * nh + 1:2 * nh + 2])
                # S[:, {g,s}] = sum over halves
                S = st.tile([128, 2], F32, tag="S")
                nc.vector.tensor_tensor(S[:], S4[:, 0:2], S4[:, 2:4], op=OP.add)
                if nonzero_attn_bias:
                    nc.vector.tensor_tensor(rg[:], rg[:], bcg[:], op=OP.add)
                    nc.vector.tensor_tensor(rs_[:], rs_[:], bcs[:], op=OP.add)
                    nc.vector.tensor_reduce(S[:, 0:1], rg[:], axis=mybir.AxisListType.X, op=OP.add)
                    nc.vector.tensor_reduce(S[:, 1:2], rs_[:], axis=mybir.AxisListType.X, op=OP.add)

                # sum of squares via ACT (fused square + accumulate)
                sq = scr.tile([128, d], F32, tag="sq")
                nc.scalar.activation(sq[:], rg[:], AT.Square, accum_out=Q[:, 0:1])
                nc.scalar.activation(sq[:], rs_[:], AT.Square, accum_out=Q[:, 1:2])

                # layernorm scalars (DVE only, quake rsqrt + 2 Newton iters)
                I32 = mybir.dt.int32
                mu = st.tile([128, 2], F32, tag="mu")
                vv = st.tile([128, 2], F32, tag="vv")
                m2 = st.tile([128, 2], F32, tag="m2")
                hh = st.tile([128, 2], I32, tag="hh")
                yy = st.tile([128, 2], F32, tag="yy")
                aa = st.tile([128, 2], F32, tag="aa")
                bb = st.tile([128, 2], F32, tag="bb")
                cx = st.tile([128, 2], F32, tag="cx")
                ri = st.tile([128, 2], F32, tag="ri")
                nb = st.tile([128, 2], F32, tag="nb")
                inv_n = 1.0 / d
                nc.vector.tensor_scalar(mu[:], S[:], inv_n, None, op0=OP.mult)
                nc.vector.tensor_scalar(vv[:], Q[:], inv_n, None, op0=OP.mult)
                nc.vector.tensor_tensor(m2[:], mu[:], mu[:], op=OP.mult)
                nc.vector.scalar_tensor_tensor(
                    vv[:], vv[:], float(LN_EPS), m2[:], op0=OP.add, op1=OP.subtract)
                nc.vector.tensor_scalar(hh[:], vv[:].bitcast(I32), 1,
                                        None, op0=OP.arith_shift_right)
                nc.vector.tensor_scalar(hh[:], hh[:], -1, 0x5F3759DF,
                                        op0=OP.mult, op1=OP.add)
                nc.vector.tensor_copy(yy[:], hh[:].bitcast(F32))
                for _ in range(2):
                    nc.vector.tensor_tensor(aa[:], yy[:], yy[:], op=OP.mult)
                    nc.vector.tensor_tensor(bb[:], aa[:], vv[:], op=OP.mult)
                    nc.vector.tensor_scalar(cx[:], bb[:], -0.5, 1.5,
                                            op0=OP.mult, op1=OP.add)
                    nc.vector.tensor_tensor(yy[:], yy[:], cx[:], op=OP.mult)
                nc.vector.scalar_tensor_tensor(
                    nb[:], mu[:], -1.0, yy[:], op0=OP.mult, op1=OP.mult)
                return dict(i=i, rg=rg, rs_=rs_, ri=yy, nb=nb)

            def phase_b(ctx):
                i, rg, rs_, ri, nb = ctx["i"], ctx["rg"], ctx["rs_"], ctx["ri"], ctx["nb"]
                rows = slice(i * 128, (i + 1) * 128)
                tg = wk.tile([128, d], F32, tag="tg")
                ts = wk.tile([128, d], F32, tag="ts")
                nc.scalar.activation(tg[:], rg[:], AT.Identity,
                                     bias=nb[:, 0:1], scale=ri[:, 0:1])
                nc.scalar.activation(ts[:], rs_[:], AT.Identity,
                                     bias=nb[:, 1:2], scale=ri[:, 1:2])
                if ln_identity:
                    yg, ys = tg, ts
                else:
                    yg = wk.tile([128, d], F32, tag="yg")
                    ys = wk.tile([128, d], F32, tag="ys")
                    nc.vector.scalar_tensor_tensor(
                        yg[:], tg[:], 1.0, gbg[:, 0:d], op0=OP.mult, op1=OP.mult)
                    nc.vector.tensor_tensor(yg[:], yg[:], gbg[:, d:2 * d], op=OP.add)
                    nc.vector.scalar_tensor_tensor(
                        ys[:], ts[:], 1.0, gbs[:, 0:d], op0=OP.mult, op1=OP.mult)
                    nc.vector.tensor_tensor(ys[:], ys[:], gbs[:, d:2 * d], op=OP.add)

                # transpose t (cast to bf16 on the PSUM->SBUF copy)
                tgT = xtp.tile([128, KS * 128], BF16, tag="tgT")
                tsT = xtp.tile([128, KS * 128], BF16, tag="tsT")
                for src_, dstT in ((tg, tgT), (ts, tsT)):
                    for h in range(2):
                        pt2 = ptrb.tile([128, 512], F32, tag="pt2")
                        for b in range(4):
                            k = h * 4 + b
                            nc.tensor.matmul(
                                pt2[:, b * 128:(b + 1) * 128],
                                src_[:, k * 128:(k + 1) * 128],
                                ident[:],
                                is_transpose=True,
                                start=(b == 0), stop=(b == 3),
                                skip_group_check=True,
                            )
                        nc.scalar.copy(dstT[:, h * 512:(h + 1) * 512], pt2[:])

                # gate matmul (bf16)
                g = io.tile([128, d], F32, tag="g")
                for nh in range(NH):
                    cols = slice(nh * 512, (nh + 1) * 512)
                    pgl = pmmg.tile([128, 512], F32, tag="pgl")
                    for k in range(KS):
                        nc.tensor.matmul(
                            pgl[:],
                            tgT[:, k * 128:(k + 1) * 128],
                            wg1[:, k * d + nh * 512: k * d + nh * 512 + 512],
                            start=(k == 0), stop=False,
                        )
                    for k in range(KS):
                        nc.tensor.matmul(
                            pgl[:],
                            tsT[:, k * 128:(k + 1) * 128],
                            wg2[:, k * d + nh * 512: k * d + nh * 512 + 512],
                            start=False, stop=(k == KS - 1),
                        )
                    if nonzero_gate_bias:
                        gl = scr.tile([128, 512], F32, tag="gl")
                        nc.vector.tensor_tensor(gl[:], pgl[:], bgg[:, cols], op=OP.add)
                        nc.scalar.activation(g[:, cols], gl[:], AT.Sigmoid)
                    else:
                        nc.scalar.activation(g[:, cols], pgl[:], AT.Sigmoid)

                # blend: out = ys + g * (yg - ys)
                dd = wk.tile([128, d], F32, tag="dd")
                pp = scr.tile([128, d], F32, tag="pp")
                outf = io.tile([128, d], F32, tag="outf")
                nc.vector.tensor_tensor(dd[:], yg[:], ys[:], op=OP.subtract)
                nc.vector.tensor_tensor(pp[:], dd[:], g[:], op=OP.mult)
                nc.vector.tensor_tensor(outf[:], pp[:], ys[:], op=OP.add)

                nc.sync.dma_start(outf_d[rows, :], outf[:])
                nc.sync.dma_start(outg_d[rows, :], g[:])

            pending = None
            for i in range(NB):
                ctx = phase_a(i)
                if pending is not None:
                    phase_b(pending)
                pending = ctx
            phase_b(pending)

    nc.compile()
    return nc


def _prep_host(inputs):
    """Host-side weight algebra. Returns dict of device arrays + flags."""
    import ml_dtypes

    f = lambda k: np.asarray(inputs[k], dtype=np.float32)
    Wv_gs, Wo_gs = f("Wv_gs"), f("Wo_gs")
    Wv_sg, Wo_sg = f("Wv_sg"), f("Wo_sg")
    bv_gs, bo_gs = f("bv_gs"), f("bo_gs")
    bv_sg, bo_sg = f("bv_sg"), f("bo_sg")
    Wg, bg = f("Wg"), f("bg")
    g_g, b_g = f("ln_g_gamma"), f("ln_g_beta")
    g_s, b_s = f("ln_s_gamma"), f("ln_s_beta")

    Wcgs = Wo_gs @ Wv_gs            # [D, D]; attn_g2s = smiles @ Wcgs.T + bcgs
    Wcsg = Wo_sg @ Wv_sg
    bcgs = Wo_gs @ bv_gs + bo_gs
    bcsg = Wo_sg @ bv_sg + bo_sg

    ln_identity = (np.all(g_g == 1.0) and np.all(b_g == 0.0)
                   and np.all(g_s == 1.0) and np.all(b_s == 0.0))

    # gate weights with gamma folded in (rows of Wg.T scale by gamma of the
    # un-gamma'd LN output t), bias collects beta contributions
    Wg1 = Wg[:, :D]                  # [D_out, D_in]
    Wg2 = Wg[:, D:]
    Wg1T = (Wg1 * g_g[None, :]).T.copy()     # [D_in, D_out]
    Wg2T = (Wg2 * g_s[None, :]).T.copy()
    bg_eff = bg + Wg1 @ b_g + Wg2 @ b_s

    out = {
        "wgs": Wcgs.T.copy(),        # [d_in, d_out] f32 (f32r-typed on device)
        "wsg": Wcsg.T.copy(),
        "wg1": Wg1T.astype(ml_dtypes.bfloat16),
        "wg2": Wg2T.astype(ml_dtypes.bfloat16),
    }
    flags = {
        "nonzero_attn_bias": bool(np.any(bcgs != 0) or np.any(bcsg != 0)),
        "nonzero_gate_bias": bool(np.any(bg_eff != 0)),
        "ln_identity": bool(ln_identity),
    }
    if flags["nonzero_attn_bias"]:
        out["bcg"] = np.tile(bcgs[None, :], (128, 1)).astype(np.float32)
        out["bcs"] = np.tile(bcsg[None, :], (128, 1)).astype(np.float32)
    if flags["nonzero_gate_bias"]:
        out["bgg"] = np.tile(bg_eff[None, :], (128, 1)).astype(np.float32)
    if not flags["ln_identity"]:
        out["gbg"] = np.tile(np.concatenate([g_g, b_g])[None, :], (128, 1)).astype(np.float32)
        out["gbs"] = np.tile(np.concatenate([g_s, b_s])[None, :], (128, 1)).astype(np.float32)
    return out, flags


def _run(inputs, rows_per_core, trace=False):
    from concourse.bass_utils import run_bass_kernel_spmd

    graph = np.ascontiguousarray(np.asarray(inputs["graph_embedding"], dtype=np.float32))
    smiles = np.ascontiguousarray(np.asarray(inputs["smiles_embedding"], dtype=np.float32))
    B = graph.shape[0]
    assert B == rows_per_core * N_CORES

    wmap, flags = _prep_host(inputs)
    key = (rows_per_core, D, flags["nonzero_attn_bias"],
           flags["nonzero_gate_bias"], flags["ln_identity"])
    if key not in _CACHE:
        _CACHE[key] = _build(rows_per_core, D, flags["nonzero_attn_bias"],
                             flags["nonzero_gate_bias"], flags["ln_identity"])
    nc = _CACHE[key]

    in_maps = []
    for c in range(N_CORES):
        rows = slice(c * rows_per_core, (c + 1) * rows_per_core)
        m = {"xg": graph[rows], "xs": smiles[rows]}
        m.update(wmap)
        in_maps.append(m)

    res = run_bass_kernel_spmd(nc, in_maps, core_ids=list(range(N_CORES)),
                               trace=trace)
    fused = np.concatenate([r["outf"] for r in res.results], axis=0)
    gate = np.concatenate([r["outg"] for r in res.results], axis=0)
    return (fused, gate), res


def kernel(**inputs):
    (fused, gate), _ = _run(inputs, B_FULL // N_CORES)
    return fused, gate
